# revision 8
# baseline (speedup 1.0000x reference)
"""Trainium2 Bass kernel for DendriticBranchLayer (top-k masked linear + shunting).

Computation (reference):
    W_e = topk32_mask(pre_w_exc) * exp(pre_w_exc)      # [4096, 8192]
    W_i = topk16_mask(pre_w_inh) * exp(pre_w_inh)      # [4096, 2048]
    e = x_exc @ W_e.T ; i = x_inh @ W_i.T
    out = e / (1 + i)                                  # [4096, 4096]

Strategy (8 NeuronCores, out-feature sharded - each core owns 512 output rows):
  - Host passes x transposed (contract-major) so the device streams perfectly
    contiguous tiles; outputs are produced transposed and un-transposed on host.
  - Per core: top-k thresholds per pre_w row via segmented DVE max8 prefilter +
    iterative max8/match_replace on the candidate set; masked-exp weight build
    (scalar_tensor_tensor fused (pw >= t) * exp(pw)); PE transpose of W into
    contract-major stationary tiles; fp32r matmuls accumulate e.T / i.T in PSUM;
    shunting division fused into the PSUM drain.
"""

import numpy as np

P = 128

CFG = dict(
    B=4096,        # batch (moving dim)
    O=512,         # out rows per core
    CE=8192,       # exc contract
    CI=2048,       # inh contract
    KE=32,
    KI=16,
    SEG_E=256,     # prefilter segment width (exc): 32 segs -> 256 candidates
    SEG_I=128,     # (inh): 16 segs -> 128 candidates
    CHUNK=512,     # weight-build chunk along contract
    BBLK=512,      # batch block (psum bank = 512 fp32)
    KTG=4,         # contract tiles per xT DMA
    NEG=-3.0e38,
    N_CORES=8,
    USE_STT=True,     # fused (pw>=t)*E via scalar_tensor_tensor
    MM_DT="float32r",  # matmul operand dtype: float32r | float32 | bfloat16
)


def build_program(cfg):
    import concourse.bacc as bacc
    import concourse.mybir as mybir
    import concourse.tile as tile

    dt = mybir.dt
    f32 = dt.float32
    mmdt = getattr(dt, cfg["MM_DT"])

    B, O, CE, CI = cfg["B"], cfg["O"], cfg["CE"], cfg["CI"]
    KE, KI = cfg["KE"], cfg["KI"]
    SEG_E, SEG_I = cfg["SEG_E"], cfg["SEG_I"]
    CHUNK, BBLK, KTG = cfg["CHUNK"], cfg["BBLK"], cfg["KTG"]
    NEG = cfg["NEG"]

    NOT = O // P               # out tiles
    KTE, KTI = CE // P, CI // P
    KT = KTE + KTI
    NBB = B // BBLK
    CANDE = (CE // SEG_E) * 8
    CANDI = (CI // SEG_I) * 8
    assert CE % CHUNK == 0 and CI % CHUNK == 0 or CI == CHUNK // 2 or CI <= CHUNK
    CHUNK_I = min(CHUNK, CI)

    nc = bacc.Bacc("TRN2", target_bir_lowering=False, debug=False,
                   num_devices=cfg["N_CORES"])

    pwe = nc.dram_tensor("pwe", [O, CE], f32, kind="ExternalInput")
    pwi = nc.dram_tensor("pwi", [O, CI], f32, kind="ExternalInput")
    xeT = nc.dram_tensor("xeT", [CE, B], mmdt, kind="ExternalInput")
    xiT = nc.dram_tensor("xiT", [CI, B], mmdt, kind="ExternalInput")
    ident = nc.dram_tensor("ident", [P, P], f32, kind="ExternalInput")
    outT = nc.dram_tensor("outT", [O, B], f32, kind="ExternalOutput")

    Exp = mybir.ActivationFunctionType.Exp
    Copy = mybir.ActivationFunctionType.Copy
    is_ge = mybir.AluOpType.is_ge
    mult = mybir.AluOpType.mult

    with tile.TileContext(nc, trace_sim=cfg.get("TRACE_SIM", False)) as tc:
        with (
            tc.tile_pool(name="persist", bufs=1) as persist,
            tc.tile_pool(name="pw", bufs=2) as pw_pool,
            tc.tile_pool(name="ebuf", bufs=2) as e_pool,
            tc.tile_pool(name="wbuf", bufs=2) as w_pool,
            tc.tile_pool(name="cand", bufs=2) as cand_pool,
            tc.tile_pool(name="v8", bufs=8) as v8_pool,
            tc.tile_pool(name="xt", bufs=2) as xt_pool,
            tc.tile_pool(name="stage", bufs=2) as stage_pool,
        ):
            id_tile = persist.tile([P, P], f32, tag="ident")
            nc.sync.dma_start(id_tile[:], ident[:])
            wt = persist.tile([P, KT, O], mmdt, tag="wt")
            t_all = persist.tile([P, 2 * NOT], f32, tag="t")

            import contextlib
            _stack = contextlib.ExitStack()
            psw_pool = _stack.enter_context(
                tc.tile_pool(name="psw", bufs=2, space="PSUM"))
            # ---------------- Phase T: per-row top-k thresholds ----------------
            def emit_threshold(ot, mat, C, K, SEG, CAND, tcol):
                nch = C // CHUNK if C >= CHUNK else 1
                chw = C // nch
                segs_per_chunk = chw // SEG
                cand = cand_pool.tile([P, CAND], f32, tag="cand")
                for ch in range(nch):
                    pwc = pw_pool.tile([P, chw], f32, tag="pw")
                    nc.sync.dma_start(
                        pwc[:], mat[ot * P:(ot + 1) * P, ch * chw:(ch + 1) * chw])
                    for s in range(segs_per_chunk):
                        gi = ch * segs_per_chunk + s
                        nc.vector.max(cand[:, gi * 8:(gi + 1) * 8],
                                      pwc[:, s * SEG:(s + 1) * SEG])
                cur = cand
                n_rounds = K // 8
                v8 = None
                for r in range(n_rounds):
                    v8 = v8_pool.tile([P, 8], f32, tag="v8")
                    nc.vector.max(v8[:], cur[:])
                    if r < n_rounds - 1:
                        nxt = cand_pool.tile([P, CAND], f32, tag="cand")
                        nc.vector.match_replace(nxt[:], v8[:], cur[:], NEG)
                        cur = nxt
                nc.vector.tensor_copy(t_all[:, tcol:tcol + 1], v8[:, 7:8])

            for ot in range(NOT):
                emit_threshold(ot, pwe, CE, KE, SEG_E, CANDE, ot)
                emit_threshold(ot, pwi, CI, KI, SEG_I, CANDI, NOT + ot)

            # ------------- Phase W: masked-exp build + PE transpose -------------
            def emit_build(ot, mat, C, kt_base, tcol, chw):
                nch = C // chw
                for ch in range(nch):
                    pwc = pw_pool.tile([P, chw], f32, tag="pw")
                    nc.sync.dma_start(
                        pwc[:], mat[ot * P:(ot + 1) * P, ch * chw:(ch + 1) * chw])
                    ebuf = e_pool.tile([P, chw], f32, tag="ebuf")
                    nc.scalar.activation(ebuf[:], pwc[:], Exp)
                    wbuf = w_pool.tile([P, chw], f32, tag="wbuf")
                    if cfg["USE_STT"]:
                        nc.vector.scalar_tensor_tensor(
                            wbuf[:], pwc[:], t_all[:, tcol:tcol + 1], ebuf[:],
                            is_ge, mult)
                    else:
                        nc.vector.tensor_scalar(
                            wbuf[:], pwc[:], t_all[:, tcol:tcol + 1], None, is_ge)
                        nc.vector.tensor_mul(wbuf[:], wbuf[:], ebuf[:])
                    nblk = chw // P
                    for g in range(0, nblk, 4):
                        gw = min(4, nblk - g)
                        pst = psw_pool.tile([P, 4 * P], f32, tag="psw")
                        for b4 in range(gw):
                            nc.tensor.transpose(
                                pst[:, b4 * P:(b4 + 1) * P],
                                wbuf[:, (g + b4) * P:(g + b4 + 1) * P],
                                id_tile[:])
                        kt0 = kt_base + ch * nblk + g
                        dst = wt[:, kt0:kt0 + gw, ot * P:(ot + 1) * P]
                        src = pst[:, :gw * P].rearrange("p (a q) -> p a q", q=P)
                        nc.scalar.activation(dst, src, Copy)

            for ot in range(NOT):
                emit_build(ot, pwe, CE, 0, ot, CHUNK)
                emit_build(ot, pwi, CI, KTE, NOT + ot, CHUNK_I)

            _stack.close()
            psm_pool = None
            _stack2 = contextlib.ExitStack()
            psm_pool = _stack2.enter_context(
                tc.tile_pool(name="psm", bufs=1, space="PSUM"))
            # ---------------- Phase M: matmuls + fused shunting ----------------
            for bb in range(NBB):
                pse = [psm_pool.tile([P, BBLK], f32, tag=f"pse{o}", name=f"pse{o}") for o in range(NOT)]
                psi = [psm_pool.tile([P, BBLK], f32, tag=f"psi{o}", name=f"psi{o}") for o in range(NOT)]
                stage_e = [None] * NOT

                def mm_part(xT, KTn, kt_base, ps):
                    for kg in range(0, KTn, KTG):
                        gw = min(KTG, KTn - kg)
                        xt = xt_pool.tile([P, KTG, BBLK], mmdt, tag="xt")
                        src = xT[kg * P:(kg + gw) * P,
                                 bb * BBLK:(bb + 1) * BBLK]
                        nc.sync.dma_start(
                            xt[:, :gw, :],
                            src.rearrange("(a p) b -> p a b", p=P))
                        for j in range(gw):
                            kt_local = kg + j
                            kt = kt_base + kt_local
                            for o in range(NOT):
                                nc.tensor.matmul(
                                    ps[o][:],
                                    wt[:, kt, o * P:(o + 1) * P],
                                    xt[:, j, :],
                                    start=(kt_local == 0),
                                    stop=(kt_local == KTn - 1),
                                )

                mm_part(xeT, KTE, 0, pse)
                # drain e early (frees banks while inh accumulates)
                for o in range(NOT):
                    stage_e[o] = stage_pool.tile([P, BBLK], f32, tag="stg_e", name=f"stg_e{o}")
                    nc.scalar.activation(stage_e[o][:], pse[o][:], Copy)
                mm_part(xiT, KTI, KTE, psi)
                for o in range(NOT):
                    onepi = stage_pool.tile([P, BBLK], f32, tag="onepi")
                    nc.vector.tensor_scalar_add(onepi[:], psi[o][:], 1.0)
                    rinv = stage_pool.tile([P, BBLK], f32, tag="rinv")
                    scratch = stage_pool.tile([P, BBLK], f32, tag="scr")
                    nc.vector.reciprocal_approx_accurate(rinv[:], onepi[:], scratch[:])
                    outb = stage_pool.tile([P, BBLK], f32, tag="scr")
                    nc.vector.tensor_mul(outb[:], stage_e[o][:], rinv[:])
                    nc.sync.dma_start(
                        outT[o * P:(o + 1) * P, bb * BBLK:(bb + 1) * BBLK],
                        outb[:])
            _stack2.close()

    nc.compile()
    return nc


_PROGRAM_CACHE = {}


def _get_program(cfg_key):
    if cfg_key not in _PROGRAM_CACHE:
        _PROGRAM_CACHE[cfg_key] = build_program(CFG)
    return _PROGRAM_CACHE[cfg_key]


def _fix_boundary_ties(pw, k):
    """Make the k-th largest of each row strictly greater than the (k+1)-th.

    jax.lax.top_k breaks exact-value ties by index (lowest first); a threshold
    mask keeps all tied values. Push the tied-but-not-selected duplicates down
    by 1 ulp - they end up masked out, so the perturbation never reaches the
    output.
    """
    part = np.partition(pw, [-k - 1, -k], axis=1)
    t, t1 = part[:, -k], part[:, -k - 1]
    bad = np.flatnonzero(t == t1)
    if bad.size == 0:
        return pw
    pw = pw.copy()
    for r in bad:
        row = pw[r]
        tv = t[r]
        dups = np.flatnonzero(row == tv)
        m = k - int((row > tv).sum())
        row[dups[m:]] = np.nextafter(tv, np.float32(-np.inf), dtype=np.float32)
    return pw


def make_in_maps(x_exc, x_inh, pre_w_exc, pre_w_inh, cfg=CFG):
    n = cfg["N_CORES"]
    O = cfg["O"]
    pre_w_exc = _fix_boundary_ties(np.asarray(pre_w_exc, np.float32), cfg["KE"])
    pre_w_inh = _fix_boundary_ties(np.asarray(pre_w_inh, np.float32), cfg["KI"])
    xeT = np.ascontiguousarray(x_exc.T)
    xiT = np.ascontiguousarray(x_inh.T)
    ident = np.eye(P, dtype=np.float32)
    in_maps = []
    for c in range(n):
        in_maps.append({
            "pwe": np.ascontiguousarray(pre_w_exc[c * O:(c + 1) * O]),
            "pwi": np.ascontiguousarray(pre_w_inh[c * O:(c + 1) * O]),
            "xeT": xeT,
            "xiT": xiT,
            "ident": ident,
        })
    return in_maps


def kernel(x_exc, x_inh, pre_w_exc, pre_w_inh):
    from concourse.bass_utils import run_bass_kernel_spmd

    nc = _get_program("main")
    in_maps = make_in_maps(x_exc, x_inh, pre_w_exc, pre_w_inh)
    res = run_bass_kernel_spmd(nc, in_maps, list(range(CFG["N_CORES"])))
    out = np.concatenate([r["outT"].T for r in res.results], axis=1)
    return np.ascontiguousarray(out.astype(np.float32))


if __name__ == "__main__":
    nc = build_program(CFG)
    print("program built + compiled OK")
